# revision 10
# baseline (speedup 1.0000x reference)
"""GNN message-passing layer (GSS GNNLayer) on 8 Trainium2 NeuronCores — v2.

Math (see reference):
    Ax   = A @ x                 (sparse COO, E edges)
    pre  = Ax @ W1.T + b1 + (A @ (Ax * x)) @ W2.T + b2
    out  = elu(pre) ; return (pre, out)

Algebraic restructure: pre = A @ G + (b1 + b2)  where
    G = x @ W1.T + (Ax * x) @ W2.T
so the dense per-node matmuls happen in pass 1 (between the SpMMs) and
pass 2's PSUM accumulation directly produces `pre`.

Distribution: row-partition by destination node; core c owns dest rows
[c*5000, (c+1)*5000). Edges are bucketed by (core, dest-block of 128,
source-quarter) on the host, sorted by source within a bucket, padded
to chunks of 128 with val=0 edges.

Tables are bf16 [*, 128] (256 B rows). Source nodes are split into 4
"quarters" by their local row range on the owning core; table k holds
rows {c*Qsize_k + (loc - Qstart_k)} for all cores c, so int16 gather
indices stay < 8*1280. Pass 1 gathers from host-built x tables; pass 2
gathers from the AllGather outputs of the same layout. The 4 AllGathers
fire as soon as their quarter of G is done, overlapping the collective
with pass-1/pass-2 compute; only the last quarter's AllGather sits on
the critical path.

SpMM per chunk of 128 edges: dma_gather 128 source rows (super-blocks of
8 dest blocks, one gather per quarter, 4 SWDGE queues), build one bf16
selection matrix S[e,d] = val[e] * (d == dloc[e]) (DVE tensor_scalar —
Pool-engine tensor ops are ucode and measure ~10x slower than the cost
model claims, so everything element-wise stays on DVE/Act), and one bf16
matmul:
    pass 1: ps[f,d]  += msl[e,f]^T S[e,d]      (Ax^T per dest block)
    pass 2: ps[d,f]  += S[e,d]^T msl[e,f]      (pre per dest block)
fin1: H^T = Ax^T * x^T (elementwise), G = x@W1T + H@W2T via
matmul(lhsT=xT_blk) + matmul(lhsT=HT) into one PSUM tile, no transposes;
PSUM->SBUF copies ride the Act engine.
fin2: bias is folded into the PSUM chain as a final matmul with a
constant partition-0 row selector (lhsT=onehot(e==0), rhs=bsum rows);
elu = max(exp(min(pre,0)) - 1, pre) in 2 DVE ops + 1 Act exp.

SPMD: one program for all 8 cores; per-(block,quarter) chunk counts are
the max over cores so the program is uniform and only the data differs.
"""

import os
import numpy as np
import ml_dtypes

BF16 = ml_dtypes.bfloat16

N = 40000
D = 128
E = 640000
NCORES = 8
NSH = N // NCORES          # 5000 dest rows per core
P = 128
NB = (NSH + P - 1) // P    # 40 dest blocks per core (last has 8 rows)
SBW = int(os.environ.get("SBW", "8"))   # blocks per gather super-block
NQ = 4                     # SWDGE queues for gathers
KQ = int(os.environ.get("KQ", "4"))     # source-range chunks (2 or 4)


def _layout(kq):
    qb = {4: (0, 10, 20, 30, 40), 2: (0, 20, 40)}[kq]
    qr = tuple(min(b * P, NSH) for b in qb)
    qsize = tuple(qr[k + 1] - qr[k] for k in range(kq))
    return qb, qr, qsize

_cache = {}


def _preprocess(adj_row, adj_col, adj_val, kq=KQ):
    """Bucket/pad edges; build per-core gather-index and S-descriptor arrays."""
    NQUART = kq
    QBLK, QR, QSIZE = _layout(kq)
    row = np.asarray(adj_row, np.int64)
    col = np.asarray(adj_col, np.int64)
    val = np.asarray(adj_val, np.float32)

    core = row // NSH
    loc = row - core * NSH
    blk = loc // P
    dloc = (loc % P).astype(np.float32)

    scs = col // NSH
    sloc = col - scs * NSH
    qr = np.asarray(QR)
    q = np.searchsorted(qr, sloc, side="right") - 1        # 0..3
    qsz = np.asarray(QSIZE)[q]
    tbl = scs * qsz + (sloc - qr[q])                        # index into table q

    key = (core * NB + blk) * NQUART + q
    nkey = NCORES * NB * NQUART
    order = np.lexsort((tbl, key))            # bucket-major, source-sorted
    sk = key[order]
    counts = np.bincount(key, minlength=nkey)
    gstart = np.concatenate([[0], np.cumsum(counts)[:-1]])
    pos = np.arange(len(sk)) - gstart[sk]     # rank within its bucket

    cnt = counts.reshape(NCORES, NB, NQUART)
    caps = np.ceil(cnt / P).astype(np.int64).max(axis=0)    # [NB, NQUART]
    caps[:, 0] = np.maximum(caps[:, 0], 1)    # every block needs >=1 chunk

    # chunk-column layout: per block, q0 chunks then q1, q2, q3
    col0 = np.zeros((NB, NQUART), np.int64)
    run = 0
    for b in range(NB):
        for k in range(NQUART):
            col0[b, k] = run
            run += caps[b, k]
    TC = int(run)
    # per-quarter chunk offsets (chunks of quarter k, ordered by block)
    qoff = np.zeros((NB, NQUART), np.int64)
    for k in range(NQUART):
        qoff[:, k] = np.concatenate([[0], np.cumsum(caps[:, k])[:-1]])
    qtot = caps.sum(axis=0)                   # chunks per quarter
    CL = [max(int(qtot[k]) * 8, 8) for k in range(NQUART)]  # idx cols (16/col)

    rowloc = np.zeros((NCORES, P, TC), np.float32)
    vv = np.zeros((NCORES, P, TC), np.float32)
    idxq = [np.zeros((NCORES, P, CL[k]), np.int16) for k in range(NQUART)]
    # debug (host-only): which (quarter, table-row) each slot points at
    dbg_q = np.zeros((NCORES, P, TC), np.int8)
    dbg_t = np.zeros((NCORES, P, TC), np.int32)

    cS = sk // (NB * NQUART)
    bS = (sk // NQUART) % NB
    qS = sk % NQUART
    dS = dloc[order]
    vS = val[order]
    tS = tbl[order]
    vb = vS.astype(BF16).astype(np.float32)

    ccol = col0[bS, qS] + pos // P
    pp = pos % P
    rowloc[cS, pp, ccol] = dS
    vv[cS, pp, ccol] = vb
    dbg_q[cS, pp, ccol] = qS
    dbg_t[cS, pp, ccol] = tS

    reps = 16 * np.arange(8)[None, :]
    for k in range(NQUART):
        m = qS == k
        if not m.any():
            continue
        qa = qoff[bS[m], k] * P + pos[m]
        idxq[k][cS[m][:, None], (qa % 16)[:, None] + reps, (qa // 16)[:, None]] = \
            tS[m].astype(np.int16)[:, None]

    return dict(caps=tuple(tuple(int(x) for x in caps[b]) for b in range(NB)),
                TC=TC, CL=tuple(CL), kq=kq,
                rowloc=rowloc, vv=vv, idxq=idxq,
                dbg_q=dbg_q, dbg_t=dbg_t)


def _build(caps, TC, CL, reps=1, kq=KQ, sbw=SBW):
    NQUART = kq
    QBLK, QR, QSIZE = _layout(kq)
    SBW = sbw
    ABL = set(os.environ.get('ABL', '').replace('+', ',').split(','))
    import concourse.bacc as bacc
    import concourse.mybir as mybir
    import concourse.tile as tile

    f32 = mybir.dt.float32
    bf16 = mybir.dt.bfloat16
    i16 = mybir.dt.int16
    Alu = mybir.AluOpType
    Act = mybir.ActivationFunctionType

    caps = np.asarray(caps, np.int64)         # [NB, NQUART]
    col0 = np.zeros((NB, NQUART), np.int64)
    run = 0
    for b in range(NB):
        for k in range(NQUART):
            col0[b, k] = run
            run += caps[b, k]
    qoff = np.zeros((NB, NQUART), np.int64)
    for k in range(NQUART):
        qoff[:, k] = np.concatenate([[0], np.cumsum(caps[:, k])[:-1]])

    NSB = NB // SBW
    # chunks per (super-block, quarter)
    nsq = [[int(caps[s * SBW:(s + 1) * SBW, k].sum()) for k in range(NQUART)]
           for s in range(NSB)]
    NROT = 8

    nc = bacc.Bacc(None, target_bir_lowering=False, num_swdge_queues=NQ)
    xq_d = [nc.declare_dram_parameter(f"xq{k}", [NCORES * QSIZE[k], D], bf16,
                                      isOutput=False) for k in range(NQUART)]
    xt_d = nc.declare_dram_parameter("xT", [D, NSH], bf16, isOutput=False)
    idx_d = [nc.declare_dram_parameter(f"idx{k}", [P, CL[k]], i16,
                                       isOutput=False) for k in range(NQUART)]
    rowloc_d = nc.declare_dram_parameter("rowloc", [P, TC], f32, isOutput=False)
    vv_d = nc.declare_dram_parameter("vv", [P, TC], f32, isOutput=False)
    w1t_d = nc.declare_dram_parameter("w1t", [D, D], bf16, isOutput=False)
    w2t_d = nc.declare_dram_parameter("w2t", [D, D], bf16, isOutput=False)
    bsum_d = nc.declare_dram_parameter("bsum", [P, D], bf16, isOutput=False)
    pre_o = nc.declare_dram_parameter("pre", [NSH, D], f32, isOutput=True)
    elu_o = nc.declare_dram_parameter("eluout", [NSH, D], f32, isOutput=True)
    g_sh = nc.dram_tensor("G_shard", [NSH, D], bf16)
    gq_d = [nc.dram_tensor(f"G_q{k}", [NCORES * QSIZE[k], D], bf16,
                           addr_space="Shared") for k in range(NQUART)]

    mb = 3
    # deep S-tile pool: the DVE pre-builds selection matrices across the
    # last AllGather's tail; 6 PSUM banks give the PE more concurrent
    # block chains ('shallow' reverts to the older, smaller pools)
    sb_ = 10 if 'shallow' in ABL else 64
    pb = 4 if 'shallow' in ABL else 6
    with tile.TileContext(nc) as tc:
        with (
            tc.tile_pool(name="const", bufs=1) as cpool,
            tc.tile_pool(name="m0", bufs=mb) as mp0,
            tc.tile_pool(name="m1", bufs=mb) as mp1,
            tc.tile_pool(name="m2", bufs=mb) as mp2,
            tc.tile_pool(name="m3", bufs=mb) as mp3,
            tc.tile_pool(name="sel", bufs=sb_) as spool,
            tc.tile_pool(name="small", bufs=3) as smp,
            tc.tile_pool(name="psum", bufs=pb, space="PSUM") as pseg,
            tc.tile_pool(name="psumg", bufs=2, space="PSUM") as pg,
        ):
            mpools = [mp0, mp1, mp2, mp3]
            iota_b = cpool.tile([P, P], bf16)
            nc.gpsimd.iota(iota_b[:], pattern=[[1, P]], base=0,
                           channel_multiplier=0,
                           allow_small_or_imprecise_dtypes=True)
            # partition-index one-hot row selector for the bias matmul
            pi_b = cpool.tile([P, P], bf16)
            nc.gpsimd.iota(pi_b[:], pattern=[[0, P]], base=0,
                           channel_multiplier=1,
                           allow_small_or_imprecise_dtypes=True)
            sel0 = cpool.tile([P, P], bf16)
            nc.vector.tensor_scalar(sel0[:], pi_b[:], 0.0, None,
                                    op0=mybir.AluOpType.is_equal)
            w1t_t = cpool.tile([D, D], bf16)
            nc.sync.dma_start(w1t_t[:], w1t_d[:])
            w2t_t = cpool.tile([D, D], bf16)
            nc.sync.dma_start(w2t_t[:], w2t_d[:])
            bsum_t = cpool.tile([P, D], bf16)
            nc.sync.dma_start(bsum_t[:], bsum_d[:])
            idx_t = []
            for k in range(NQUART):
                t = cpool.tile([P, CL[k]], i16, tag=f"idx{k}")
                nc.sync.dma_start(t[:], idx_d[k][:])
                idx_t.append(t)
            rowloc_t = cpool.tile([P, TC], f32)
            nc.sync.dma_start(rowloc_t[:], rowloc_d[:])
            vv_t = cpool.tile([P, TC], f32)
            nc.sync.dma_start(vv_t[:], vv_d[:])
            # x^T shard, split into per-quarter tiles (fewer readers each)
            xqt_t = []
            for k in range(NQUART):
                t = cpool.tile([D, QSIZE[k]], bf16, tag=f"xqt{k}")
                nc.sync.dma_start(t[:], xt_d[:, QR[k]:QR[k + 1]])
                xqt_t.append(t)
            # rotating weight-tile copies for the per-block dense matmuls
            w1r, w2r, s0r, bsr = [], [], [], []
            for k in range(NROT):
                t1 = cpool.tile([D, D], bf16, tag=f"w1r{k}")
                nc.vector.tensor_copy(t1[:], w1t_t[:])
                w1r.append(t1)
                t2 = cpool.tile([D, D], bf16, tag=f"w2r{k}")
                nc.vector.tensor_copy(t2[:], w2t_t[:])
                w2r.append(t2)
                t3 = cpool.tile([P, P], bf16, tag=f"s0r{k}")
                nc.vector.tensor_copy(t3[:], sel0[:])
                s0r.append(t3)
                t4 = cpool.tile([P, D], bf16, tag=f"bsr{k}")
                nc.vector.tensor_copy(t4[:], bsum_t[:])
                bsr.append(t4)

            qctr = [0]
            sctr = [0]

            def run_once():
                def spmm_pass(tables, finalize, pass1):
                    # quarter -> AllGather emitter, deferred to next sb
                    pending_ag = []
                    for s in range(NSB):
                        b0 = s * SBW
                        mts = [None] * NQUART
                        for k in range(NQUART):
                            n = nsq[s][k]
                            if n == 0:
                                continue
                            mt = mpools[k].tile([P, n, D], bf16, tag=f"m{k}")
                            mts[k] = mt
                            if 'nogather' in ABL:
                                nc.sync.dma_start(mt[:, 0, :], tables[k][0:P, :])
                            else:
                                nc.gpsimd.dma_gather(
                                    out_ap=mt[:], in_ap=tables[k][:],
                                    idxs_ap=idx_t[k][:, qoff[b0, k] * 8:
                                                     (qoff[b0, k] + n) * 8],
                                    num_idxs=n * P, num_idxs_reg=n * P,
                                    elem_size=D, single_packet=False,
                                    queue_num=qctr[0] % NQ)
                                qctr[0] += 1
                        while pending_ag:
                            pending_ag.pop(0)()
                        for i in range(SBW):
                            b = b0 + i
                            ps = pseg.tile([P, P], f32, tag="seg")
                            tot = int(caps[b].sum())
                            done = 0
                            for k in range(NQUART):
                                goff = int(caps[b0:b, k].sum())
                                for j in range(int(caps[b, k])):
                                    c = int(col0[b, k]) + j
                                    msl = mts[k][:, goff + j, :]
                                    if 'noseg' in ABL:
                                        done += 1
                                        continue
                                    sv = spool.tile([P, P], bf16, tag="S")
                                    # Pool-engine tensor ops are ucode — far
                                    # slower on HW than the cost model says.
                                    eng = nc.gpsimd if (
                                        'pool3' in ABL
                                        and sctr[0] % 3 == 2) else nc.vector
                                    sctr[0] += 1
                                    eng.tensor_scalar(
                                        sv[:], iota_b[:],
                                        rowloc_t[:, c:c + 1], vv_t[:, c:c + 1],
                                        op0=Alu.is_equal, op1=Alu.mult)
                                    if 'nomm' in ABL:
                                        done += 1
                                        continue
                                    first = done == 0
                                    last = done == tot - 1
                                    if pass1:
                                        nc.tensor.matmul(ps[:], lhsT=msl,
                                                         rhs=sv[:],
                                                         start=first, stop=last)
                                    else:
                                        # bias matmul below carries stop=True
                                        nc.tensor.matmul(ps[:], lhsT=sv[:],
                                                         rhs=msl,
                                                         start=first,
                                                         stop=False)
                                    done += 1
                            if not pass1 and done == tot and \
                                    'nomm' not in ABL and 'noseg' not in ABL:
                                nc.tensor.matmul(ps[:], lhsT=s0r[b % NROT][:],
                                                 rhs=bsr[b % NROT][:],
                                                 start=False, stop=True)
                            finalize(b, ps)
                        if pass1:
                            for k in range(NQUART):
                                if QBLK[k + 1] - 1 in range(b0, b0 + SBW):
                                    kk = k

                                    def emit(kk=kk):
                                        if 'noag' in ABL:
                                            return
                                        nc.gpsimd.collective_compute(
                                            "AllGather", Alu.bypass,
                                            replica_groups=[list(range(NCORES))],
                                            ins=[g_sh[QR[kk]:QR[kk + 1], :]],
                                            outs=[gq_d[kk][:]])
                                    if s == NSB - 1:
                                        emit()
                                    else:
                                        pending_ag.append(emit)
                    while pending_ag:
                        pending_ag.pop(0)()

                def fin1(b, ps):
                    if 'nofin' in ABL:
                        return
                    rows = min(P, NSH - b * P)
                    kb = b // (NB // NQUART)
                    l0 = b * P - QR[kb]
                    ht = smp.tile([P, P], bf16, tag="ht")
                    if 'nomm' in ABL or 'noseg' in ABL:
                        nc.vector.memset(ht[:], 0.0)
                    elif 'dht' in ABL:
                        # DVE reads Ax^T straight from PSUM
                        nc.vector.tensor_tensor(ht[:, :rows], ps[:, :rows],
                                                xqt_t[kb][:, l0:l0 + rows],
                                                op=Alu.mult)
                    else:
                        axb = smp.tile([P, P], bf16, tag="axb")
                        nc.scalar.activation(axb[:], ps[:], Act.Copy)
                        nc.vector.tensor_tensor(ht[:, :rows], axb[:, :rows],
                                                xqt_t[kb][:, l0:l0 + rows],
                                                op=Alu.mult)
                    gp = pg.tile([P, P], f32, tag="gp")
                    nc.tensor.matmul(gp[:rows, :],
                                     lhsT=xqt_t[kb][:, l0:l0 + rows],
                                     rhs=w1r[b % NROT][:],
                                     start=True, stop=False)
                    nc.tensor.matmul(gp[:rows, :], lhsT=ht[:, :rows],
                                     rhs=w2r[b % NROT][:],
                                     start=False, stop=True)
                    gb = smp.tile([P, P], bf16, tag="gb")
                    nc.scalar.activation(gb[:rows, :], gp[:rows, :], Act.Copy)
                    nc.sync.dma_start(g_sh[b * P:b * P + rows, :],
                                      gb[:rows, :])

                def fin2(b, ps):
                    if 'nofin' in ABL:
                        return
                    rows = min(P, NSH - b * P)
                    pre_sb = smp.tile([P, P], f32, tag="presb")
                    if 'nomm' in ABL or 'noseg' in ABL:
                        nc.vector.memset(pre_sb[:], 0.0)
                    else:
                        nc.scalar.activation(pre_sb[:], ps[:], Act.Copy)
                    nc.sync.dma_start(pre_o[b * P:b * P + rows, :],
                                      pre_sb[:rows, :])
                    n1 = smp.tile([P, P], f32, tag="n1")
                    nc.vector.tensor_scalar_min(n1[:], pre_sb[:], 0.0)
                    ex = smp.tile([P, P], f32, tag="ex")
                    nc.scalar.activation(ex[:], n1[:], Act.Exp)
                    # elu = max(exp(min(x,0)) - 1, x)
                    elu = smp.tile([P, P], f32, tag="elu")
                    nc.vector.scalar_tensor_tensor(elu[:], ex[:], -1.0,
                                                   pre_sb[:],
                                                   op0=Alu.add, op1=Alu.max)
                    nc.sync.dma_start(elu_o[b * P:b * P + rows, :],
                                      elu[:rows, :])

                spmm_pass(xq_d, fin1, pass1=True)
                if 'p2fromx2' in ABL:
                    spmm_pass(xq_d, fin2, pass1=False)
                else:
                    spmm_pass(gq_d, fin2, pass1=False)

            for _ in range(reps):
                run_once()

    nc.compile()
    return nc


def _get_program(pp, reps=1, sbw=None):
    sbw = SBW if sbw is None else sbw
    key = (pp["caps"], reps, pp["kq"], sbw, os.environ.get("ABL", ""))
    if key not in _cache:
        _cache[key] = _build(pp["caps"], pp["TC"], list(pp["CL"]), reps=reps,
                             kq=pp["kq"], sbw=sbw)
    return _cache[key]


def _in_maps(pp, features, W1, b1, W2, b2):
    NQUART = pp["kq"]
    QBLK, QR, QSIZE = _layout(NQUART)
    feats = np.ascontiguousarray(np.asarray(features, np.float32))
    xb = feats.astype(BF16)                       # [N, D] bf16
    xr = xb.reshape(NCORES, NSH, D)
    xq = [np.ascontiguousarray(xr[:, QR[k]:QR[k + 1]].reshape(-1, D))
          for k in range(NQUART)]
    w1t = np.ascontiguousarray(np.asarray(W1, np.float32).T.astype(BF16))
    w2t = np.ascontiguousarray(np.asarray(W2, np.float32).T.astype(BF16))
    bsum = np.tile((np.asarray(b1, np.float32)
                    + np.asarray(b2, np.float32)).astype(BF16)[None, :],
                   (P, 1))
    maps = []
    for c in range(NCORES):
        m = {
            "xT": np.ascontiguousarray(xb[c * NSH:(c + 1) * NSH].T),
            "rowloc": pp["rowloc"][c],
            "vv": pp["vv"][c],
            "w1t": w1t,
            "w2t": w2t,
            "bsum": bsum,
        }
        for k in range(NQUART):
            m[f"xq{k}"] = xq[k]
            m[f"idx{k}"] = pp["idxq"][k][c]
        maps.append(m)
    return maps


def kernel(features, adj_row, adj_col, adj_val, W1, b1, W2, b2):
    from concourse.bass_utils import run_bass_kernel_spmd

    pp = _preprocess(adj_row, adj_col, adj_val)
    nc = _get_program(pp)
    maps = _in_maps(pp, features, W1, b1, W2, b2)
    res = run_bass_kernel_spmd(nc, maps, list(range(NCORES)))
    pre = np.concatenate([res.results[c]["pre"] for c in range(NCORES)], axis=0)
    out = np.concatenate([res.results[c]["eluout"] for c in range(NCORES)],
                         axis=0)
    return (pre, out)
